# revision 8
# baseline (speedup 1.0000x reference)
"""Distributed Trainium2 Bass kernel for the contextual-attention module.

Strategy: data-parallel over batch (2 samples x 4 cores); within a sample
the L=4096 patch/kernel axis is sharded 4 ways (1024 kernels per core).

v3 restructure (from the v2 baseline, driven by the NTFF trace):
  * GEMM1 tap-pairs read boxbf8 DIRECTLY via hand-built strided APs
    (pair = [128][2 taps @ delta][8 rows @ 66][64 cols]) - the nine 512KB
    SBUF->SBUF pre-shift DMAs (54us serial stall) are gone.
  * The static softmax reference M~ rides in the 5th DoubleRow pass's
    padding slot as a rank-1 term: stationary block 9 = -8 on channel
    row 0, moving block = m~/8 (fp8, per-column).  The per-tile vector
    subtract (64 x [128,512]) and the msbc broadcast are gone; exp reads
    the score PSUM directly.
  * Big zero-fills via bitcast-u32 views (4x fewer elements) instead of
    35us of fp8 gpsimd memsets.
  * Kernel norms via box-filter-of-squares (2 colsum matmuls) instead of
    18 Square+matmul pairs; kern_lc8 built by fp8 PE transposes of the
    already-normalized kernT8 (batched 8-per-PSUM-bank, one copy per tap).
  * Blend maps im = (1-m)*C1 and mf = fg*m/G precomputed once; GEMM2
    accumulates all 9 taps in ONE PSUM bank; blend = mul (vector) +
    add (gpsimd); ReduceScatter + output store per chunk.
"""

import os
import sys
import types

for _p in ("/opt/trn_rl_repo",):
    if os.path.isdir(_p) and _p not in sys.path:
        sys.path.append(_p)


def _ensure_axon_hooks():
    try:
        import antenv.axon_hooks  # noqa: F401
        return
    except Exception:
        pass
    try:
        import antenv
        mod = types.ModuleType("antenv.axon_hooks")
        mod._hook = None

        def set_axon_ntff_profile_hook(hook):
            mod._hook = hook

        def get_axon_ntff_profile_hook():
            return mod._hook

        mod.set_axon_ntff_profile_hook = set_axon_ntff_profile_hook
        mod.get_axon_ntff_profile_hook = get_axon_ntff_profile_hook
        sys.modules["antenv.axon_hooks"] = mod
        antenv.axon_hooks = mod
    except Exception:
        pass


_ensure_axon_hooks()

import numpy as np  # noqa: E402

NCH = 128           # channels
W = H = 64          # spatial
S = W * H           # 4096 spatial positions
B = 2               # batch
G = 4               # cores per sample
NCORES = 8
LS = S // G         # kernels per core (1024)
LT = LS // 128      # l-tiles per core (8)
ROWS = 8            # patch-center rows per chunk
CS = ROWS * H       # spatial chunk (512)
NCHUNK = W // ROWS  # 8 chunks

KS = 16.0           # kernel scale folded into kernT8 (power of 2)
AS = 128.0          # attn scale folded into fac (fp8e4 max normal ~240)
BETA = 0.6          # softmax reference = BETA * Cauchy bound
QSLOP = 1.16        # fp8 quantization slop on the Cauchy bound
MS_C = 8.0          # m~ rides fp8 as m~/MS_C; stationary row carries -MS_C
C1 = 1.0 / (9.0 * KS * AS)   # blend scale for the GEMM2 PSUM

DC = 4              # combine lookahead (chunks); gemm2 lookahead = DC+1

_CACHE = {}
LAST_EXEC_TIME_NS = None


def _build():
    from concourse import bacc, tile, mybir
    from concourse.ap import AP
    from concourse.masks import make_identity

    F32 = mybir.dt.float32
    BF = mybir.dt.bfloat16
    F8 = mybir.dt.float8e4
    U32 = mybir.dt.uint32
    Alu = mybir.AluOpType
    Act = mybir.ActivationFunctionType
    AxX = mybir.AxisListType.X
    DR = mybir.MatmulPerfMode.DoubleRow

    nc = bacc.Bacc("TRN2", target_bir_lowering=False, debug=False,
                   num_devices=NCORES)

    fg_ext = nc.dram_tensor("fg", [NCH, S], F32, kind="ExternalInput")
    fgband_ext = nc.dram_tensor("fgband", [NCH, 18 * H], F32,
                                kind="ExternalInput")
    mask_ext = nc.dram_tensor("mask", [1, S], F32, kind="ExternalInput")
    mband_ext = nc.dram_tensor("maskband", [1, 18 * H], F32,
                               kind="ExternalInput")
    out_ext = nc.dram_tensor("out", [NCH // G, S], BF, kind="ExternalOutput")

    groups = [[0, 1, 2, 3], [4, 5, 6, 7]]
    # DoubleRow tap pairs (d0,d1) as (base_offset, delta) in the 66-wide
    # padded boxbf8 element space; tap d=(dy,dx) sits at offset dy*66+dx.
    PAIRS = [(0, 1), (2, 64), (67, 1), (132, 1)]   # taps (0,1),(2,3),(4,5),(6,7)
    MSF = KS * BETA * QSLOP

    with tile.TileContext(nc) as tc:
        with tc.tile_pool(name="const", bufs=1) as cpool, \
             tc.tile_pool(name="pers", bufs=1) as pers, \
             tc.tile_pool(name="ets", bufs=DC + 1) as etp, \
             tc.tile_pool(name="st", bufs=2) as st, \
             tc.tile_pool(name="bl", bufs=2) as bl, \
             tc.tile_pool(name="dram", bufs=3, space="DRAM") as dram, \
             tc.tile_pool(name="dramP", bufs=1, space="DRAM") as dramP:

            ident8 = cpool.tile([128, 128], F8, tag="id8")
            make_identity(nc, ident8[:])
            ones_cb = cpool.tile([128, 1], BF, tag="ones")
            nc.gpsimd.memset(ones_cb[:], 1.0)

            # ---------------- persistent tensors ----------------
            boxbf8 = pers.tile([NCH, 66, 66], F8, tag="boxbf8")
            a_flat = pers.tile([128, LT * 66 * 66], F8, tag="afull")
            af4 = a_flat[:].rearrange("c (t y x) -> c t y x", t=LT, y=66)
            kt_flat = pers.tile([NCH, 10 * LS], F8, tag="kernT8")
            kv = kt_flat[:].rearrange("c (d l) -> c d l", d=10)
            klc_flat = pers.tile([128, 9 * LT * 128], F8, tag="kernlc8")
            klcv = klc_flat[:].rearrange("l (d t c) -> l d t c", d=9, t=LT)
            mp_flat = pers.tile([128, 2 * 64 * 64], F8, tag="mpair")
            mpv = mp_flat[:].rearrange("c (b y x) -> c b y x", b=2, y=64)
            mrow_bf = pers.tile([1, S], BF, tag="mrowbf")

            bar_in = dramP.tile([16], F32, tag="bari")
            bar_out = dramP.tile([16 * NCORES], F32, tag="baro")
            bar2_in = dramP.tile([16], F32, tag="bari2")
            bar2_out = dramP.tile([16 * G], F32, tag="baro2")
            ssq_d = dramP.tile([66 * 66], F32, tag="ssqd")

            # big zero fills, issued first (no input deps; overlap DMAs):
            # a_full is fully zeroed once - combine() writes every interior
            # cell over the 8 chunks, the 1-px frame must stay 0 for GEMM2.
            # Split across three engines via u32 views.
            au = a_flat[:].bitcast(U32)
            third = 8712 // 3
            nc.vector.memset(au[:, 0:third], 0)
            nc.scalar.memzero(au[:, third:2 * third])
            nc.gpsimd.memset(au[:, 2 * third:8712], 0)
            # m~ moving block: channels 1..127 must be zero (channel 0 gets
            # the m~ row via DMA afterwards)
            nc.scalar.memzero(mp_flat[:, 64 * 64:2 * 64 * 64])
            # stationary block 9: zero except channel row 0 = -MS_C
            nc.scalar.memzero(kt_flat[:, 9 * LS:10 * LS])
            nc.gpsimd.memset(kv[0:1, 9, :], -MS_C)

            # ------------ prep: IO pool spans both stages ---------------
            pio_cm = tc.tile_pool(name="prepio", bufs=1)
            pio = pio_cm.__enter__()
            fgtmp = pio.tile([NCH, W, H], F32, tag="fgtmp")
            m2d = pio.tile([64, 64], F32, tag="m2d")
            m2d8 = pio.tile([64, 64], BF, tag="m2d8")
            mband_row = pio.tile([1, 18 * H], F32, tag="mbandrow")
            fgband_sb = pio.tile([NCH, 18, H], F32, tag="fgband")
            # warm-up barrier: DRAM->DRAM trigger copies straight from the
            # input, so the rendezvous starts immediately and the gpsimd
            # trigger writes don't wait on SBUF loads
            nc.sync.dma_start(bar_in[:], mband_ext[0:1, 0:16])
            nc.gpsimd.collective_compute(
                "AllGather", Alu.bypass,
                replica_groups=[list(range(NCORES))],
                ins=[bar_in.opt()], outs=[bar_out.opt()])
            nc.sync.dma_start(bar2_in[:], mband_ext[0:1, 16:32])
            nc.gpsimd.collective_compute(
                "AllGather", Alu.bypass, replica_groups=groups,
                ins=[bar2_in.opt()], outs=[bar2_out.opt()])
            nc.sync.dma_start(mband_row[:], mband_ext[:])
            nc.sync.dma_start(
                fgband_sb[:],
                fgband_ext[:].rearrange("c (r x) -> c r x", r=18))
            fg3 = fg_ext[:].rearrange("c (y x) -> c y x", y=W)
            nc.scalar.dma_start(fgtmp[:, 0:32, :], fg3[:, 0:32, :])
            nc.gpsimd.dma_start(fgtmp[:, 32:64, :], fg3[:, 32:64, :])
            nc.sync.dma_start(m2d[:],
                              mask_ext[:].rearrange("o (p x) -> (o p) x",
                                                    p=64))
            nc.vector.tensor_copy(m2d8[:], m2d[:])

            # ------------ prep stage 1: kernels + norms (band path) -----
            with tc.tile_pool(name="prep1", bufs=1) as prep, \
                 tc.tile_pool(name="psP1", bufs=2, space="PSUM") as psP:
                # masked band -> bgbandp (zero-padded cols), bf16
                mband_bc = prep.tile([NCH, 18 * H], F32, tag="mbandbc")
                nc.gpsimd.partition_broadcast(mband_bc[:], mband_row[:])
                bgbandp = prep.tile([NCH, 18, 66], BF, tag="bgbandp")
                nc.scalar.memzero(bgbandp[:])
                nc.vector.tensor_mul(
                    bgbandp[:, :, 1:65], fgband_sb[:],
                    mband_bc[:].rearrange("c (r x) -> c r x", r=18))

                # kernel norms via box-filter of squares
                bgsq = prep.tile([NCH, 18, 66], BF, tag="bgsq")
                nc.scalar.activation(
                    bgsq[:].rearrange("c r x -> c (r x)"),
                    bgbandp[:].rearrange("c r x -> c (r x)"), Act.Square)
                hsum = prep.tile([NCH, 18, 64], BF, tag="hsum")
                nc.vector.tensor_add(hsum[:], bgsq[:, :, 0:64],
                                     bgsq[:, :, 1:65])
                nc.vector.tensor_add(hsum[:], hsum[:], bgsq[:, :, 2:66])
                nrm2 = prep.tile([NCH, 16, 64], BF, tag="mbandbc")
                nc.vector.tensor_add(nrm2[:], hsum[:, 0:16], hsum[:, 1:17])
                nc.vector.tensor_add(nrm2[:], nrm2[:], hsum[:, 2:18])
                # reciprocal on a [1, N] row is ~6.5us (single partition);
                # bounce through DRAM to a [128, 8] layout instead
                n2_row = prep.tile([1, LS], F32, tag="normrow")
                for h in range(2):
                    ps_n = psP.tile([1, 512], F32, tag="psn")
                    nc.tensor.matmul(ps_n[:], ones_cb[:],
                                     nrm2[:, 8 * h:8 * h + 8, :],
                                     start=True, stop=True)
                    nc.scalar.activation(n2_row[:, 512 * h:512 * (h + 1)],
                                         ps_n[:], Act.Identity)
                n2_d = dramP.tile([LS], F32, tag="n2d")
                nc.sync.dma_start(n2_d[:], n2_row[:])
                n2col = prep.tile([128, LT], F32, tag="n2col")
                nc.sync.dma_start(
                    n2col[:], n2_d[:].rearrange("(t p) -> p t", p=128))
                rcol = prep.tile([128, LT], F32, tag="rcol")
                nc.vector.reciprocal(rcol[:], n2col[:])
                rcol8 = prep.tile([128, LT], BF, tag="rcol8")
                nc.scalar.activation(rcol8[:], rcol[:], Act.Sqrt,
                                     scale=KS * KS)
                rn_d = dramP.tile([LS], BF, tag="rnd")
                nc.sync.dma_start(rn_d[:].rearrange("(t p) -> p t", p=128),
                                  rcol8[:])
                rnorm_bf = prep.tile([1, LS], BF, tag="rnormbf")
                nc.sync.dma_start(rnorm_bf[:], rn_d[:])
                rnorm_bc = prep.tile([NCH, LS], BF, tag="rnormbc")
                nc.gpsimd.partition_broadcast(rnorm_bc[:], rnorm_bf[:])
                rnorm3 = rnorm_bc[:].rearrange("c (r x) -> c r x", r=16)

                # normalized fp8 kernels, one strided mul per tap
                for d in range(9):
                    dy, dx = d // 3, d % 3
                    eng = nc.vector if d < 5 else nc.gpsimd
                    eng.tensor_mul(
                        kv[:, d, :].rearrange("c (r x) -> c r x", r=16),
                        bgbandp[:, dy:dy + 16, dx:dx + 64], rnorm3)

            # ------------ prep stage 2: boxfeat + M~ --------------------
            with tc.tile_pool(name="prep2", bufs=1) as prep, \
                 tc.tile_pool(name="psP2", bufs=2, space="PSUM") as psP:
                nc.sync.dma_start(mrow_bf[:], m2d8[:])

                # box-filtered feature map (3x3 sums, 66x66 padded grid)
                boxbf = prep.tile([NCH, 66, 66], BF, tag="boxbf")
                hp = prep.tile([NCH, W, 63], BF, tag="hvp")
                nc.vector.tensor_add(hp[:, 0:18, :], fgtmp[:, 0:18, 0:63],
                                     fgtmp[:, 0:18, 1:64])
                tmpH = prep.tile([NCH, W, 66], BF, tag="tmpH")
                nc.vector.tensor_add(tmpH[:, 0:18, 2:64], hp[:, 0:18, 0:62],
                                     fgtmp[:, 0:18, 2:64])
                nc.vector.tensor_add(hp[:, 18:64, :], fgtmp[:, 18:64, 0:63],
                                     fgtmp[:, 18:64, 1:64])
                nc.vector.tensor_add(tmpH[:, 18:64, 2:64], hp[:, 18:64, 0:62],
                                     fgtmp[:, 18:64, 2:64])
                nc.vector.tensor_copy(tmpH[:, :, 0:1], fgtmp[:, :, 0:1])
                nc.vector.tensor_copy(tmpH[:, :, 1:2], hp[:, :, 0:1])
                nc.vector.tensor_copy(tmpH[:, :, 64:65], hp[:, :, 62:63])
                nc.vector.tensor_copy(tmpH[:, :, 65:66], fgtmp[:, :, 63:64])
                vp = prep.tile([NCH, 63, 66], BF, tag="hvp")
                nc.vector.tensor_add(vp[:, 0:17, :], tmpH[:, 0:17, :],
                                     tmpH[:, 1:18, :])
                nc.vector.tensor_add(boxbf[:, 2:18, :], vp[:, 0:16, :],
                                     tmpH[:, 2:18, :])
                nc.vector.tensor_copy(boxbf[:, 0:1, :], tmpH[:, 0:1, :])
                nc.vector.tensor_copy(boxbf[:, 1:2, :], vp[:, 0:1, :])
                nc.vector.tensor_add(vp[:, 17:63, :], tmpH[:, 17:63, :],
                                     tmpH[:, 18:64, :])
                nc.vector.tensor_add(boxbf[:, 18:64, :], vp[:, 16:62, :],
                                     tmpH[:, 18:64, :])
                nc.vector.tensor_copy(boxbf[:, 64:65, :], vp[:, 62:63, :])
                nc.vector.tensor_copy(boxbf[:, 65:66, :], tmpH[:, 63:64, :])
                nc.scalar.activation(
                    boxbf8[:].rearrange("c y x -> c (y x)"),
                    boxbf[:].rearrange("c y x -> c (y x)"), Act.Identity)

                # m~ moving block 0 = tap-8 window of boxbf8
                nc.vector.tensor_copy(mpv[:, 0, :, :], boxbf8[:, 2:66, 2:66])

                # ---- static softmax reference M~ (Cauchy bound) ----
                boxsq = prep.tile([NCH, 66, 66], BF, tag="boxsq")
                nc.vector.tensor_mul(
                    boxsq[:].rearrange("c y x -> c (y x)"),
                    boxbf[:].rearrange("c y x -> c (y x)"),
                    boxbf[:].rearrange("c y x -> c (y x)"))
                r0s = list(range(0, 63, 7)) + [63]
                for r0 in r0s:
                    n = min(7, 66 - r0)
                    ps_q = psP.tile([1, 512], F32, tag="psq")
                    nc.tensor.matmul(
                        ps_q[:, 0:n * 66], ones_cb[:],
                        boxsq[:, r0:r0 + n, :],
                        start=True, stop=True)
                    sq_sb = prep.tile([1, 512], F32, tag="sqsb")
                    nc.scalar.activation(sq_sb[:, 0:n * 66],
                                         ps_q[:, 0:n * 66], Act.Identity)
                    nc.sync.dma_start(ssq_d[r0 * 66:(r0 + n) * 66],
                                      sq_sb[:, 0:n * 66])
                sshift = []
                for i in range(3):
                    s_i = prep.tile([64, 66], F32, tag=f"sshift{i}")
                    nc.sync.dma_start(
                        s_i[:], ssq_d[i * 66:(i + 64) * 66]
                        .rearrange("(p x) -> p x", p=64))
                    sshift.append(s_i)
                vq = prep.tile([64, 66], F32, tag="vq")
                nc.vector.tensor_add(vq[:], sshift[0][:], sshift[1][:])
                nc.vector.tensor_add(vq[:], vq[:], sshift[2][:])
                hq = prep.tile([64, 64], F32, tag="hq")
                nc.vector.tensor_add(hq[:], vq[:, 0:64], vq[:, 1:65])
                nc.vector.tensor_add(hq[:], hq[:], vq[:, 2:66])
                ms8 = prep.tile([64, 64], F8, tag="ms8")
                msf8 = MSF / MS_C
                nc.scalar.activation(ms8[:], hq[:], Act.Sqrt,
                                     scale=msf8 * msf8)
                nc.sync.dma_start(mpv[0:1, 1, :, :], ms8[:])

            pio_cm.__exit__(None, None, None)

            # ---------------- pipelined chunk loop ----------------
            bb = boxbf8[:]
            bb_part = list(bb.ap[0])     # [partition_stride, 128]

            def g1_moving(k, p):
                base, delta = PAIRS[p]
                return AP(bb.tensor, bb.offset + base + k * ROWS * 66,
                          [bb_part, [delta, 2], [66, ROWS], [1, 64]])

            with tc.tile_pool(name="psA", bufs=3, space="PSUM") as psA, \
                 tc.tile_pool(name="psG2", bufs=1, space="PSUM") as psG2, \
                 tc.tile_pool(name="psZ", bufs=2, space="PSUM") as psZ, \
                 tc.tile_pool(name="psT", bufs=2, space="PSUM") as psT:

                def emit_g1(k):
                    """fp8 DR score GEMM (incl. rank-1 m~ tap) + exp."""
                    ets = []
                    for t in range(LT):
                        ps = psA.tile([128, CS], F32, tag="psA")
                        for p in range(4):
                            nc.tensor.matmul(
                                ps[:],
                                kv[:, 2 * p:2 * p + 2,
                                   t * 128:(t + 1) * 128],
                                g1_moving(k, p),
                                start=(p == 0), stop=False, perf_mode=DR)
                        nc.tensor.matmul(
                            ps[:],
                            kv[:, 8:10, t * 128:(t + 1) * 128],
                            mpv[:, :, k * ROWS:(k + 1) * ROWS, :],
                            start=False, stop=True, perf_mode=DR)
                        et = etp.tile([128, CS], BF, tag=f"et{t}")
                        nc.scalar.activation(et[:], ps[:], Act.Exp,
                                             scale=1.0 / KS)
                        ets.append(et)
                    return ets

                def emit_transp(d):
                    """kern_lc8 tap d: 8 fp8 PE transposes + one copy.
                    fp8 transpose output must land at element step 2."""
                    pt = psT.tile([128, LT, 128, 2], F8, tag="psT")
                    for t in range(LT):
                        nc.tensor.transpose(
                            pt[:, t, :, 0], kv[:, d, t * 128:(t + 1) * 128],
                            ident8[:])
                    nc.vector.tensor_copy(klcv[:, d, :, :], pt[:, :, :, 0])

                def emit_z_ag(k, ets):
                    ps_z = psZ.tile([1, CS], F32, tag="psZ")
                    for t in range(LT):
                        nc.tensor.matmul(ps_z[:], ones_cb[:], ets[t][:],
                                         start=(t == 0), stop=(t == LT - 1))
                    s_row = st.tile([1, CS], F32, tag="srow")
                    nc.scalar.activation(s_row[:], ps_z[:], Act.Identity)
                    ag_in = dram.tile([CS], F32, tag="agi")
                    nc.sync.dma_start(ag_in[:], s_row[:])
                    ag_out = dram.tile([CS * G], F32, tag="ago")
                    nc.gpsimd.collective_compute(
                        "AllGather", Alu.bypass, replica_groups=groups,
                        ins=[ag_in.opt()], outs=[ag_out.opt()])
                    return ag_out

                def emit_combine(k, ets, ag_out):
                    """global Z -> fac = AS/Z broadcast; a8 = et * fac."""
                    zz = st.tile([32, G, CS // 32], F32, tag="zz")
                    nc.sync.dma_start(
                        zz[:], ag_out[:].rearrange("(r p i) -> p r i",
                                                   r=G, p=32))
                    gs = st.tile([32, CS // 32], F32, tag="gs")
                    nc.vector.tensor_reduce(
                        gs[:], zz[:].rearrange("p r i -> p i r"), AxX,
                        Alu.add)
                    rg = st.tile([32, CS // 32], F32, tag="rg")
                    nc.vector.reciprocal(rg[:], gs[:])
                    fac32 = st.tile([32, CS // 32], BF, tag="fac32")
                    nc.vector.tensor_scalar_mul(fac32[:], rg[:], AS)
                    fac_row = st.tile([1, CS], BF, tag="facrow")
                    nc.sync.dma_start(fac_row[:], fac32[:])
                    fac_bc = st.tile([128, CS], BF, tag="facbc")
                    nc.gpsimd.partition_broadcast(fac_bc[:], fac_row[:])
                    fb3 = fac_bc[:].rearrange("p (y x) -> p y x", y=ROWS)
                    for t in range(LT):
                        nc.vector.tensor_mul(
                            af4[:, t, 1 + k * ROWS:1 + (k + 1) * ROWS, 1:65],
                            ets[t][:].rearrange("p (y x) -> p y x", y=ROWS),
                            fb3)

                cv_pend = {}

                def emit_g2_blend(j):
                    """fp8 transpose-conv, all 9 taps in one PSUM bank,
                    then blend; ReduceScatter every two chunks."""
                    ps2 = psG2.tile([128, CS], F32, tag="psG2")
                    for d in range(9):
                        dy, dx = d // 3, d % 3
                        sy0 = j * ROWS + 2 - dy
                        sx0 = 2 - dx
                        for p in range(4):
                            nc.tensor.matmul(
                                ps2[:],
                                klcv[:, d, 2 * p:2 * p + 2, :],
                                af4[:, 2 * p:2 * p + 2,
                                    sy0:sy0 + ROWS, sx0:sx0 + 64],
                                start=(d == 0 and p == 0),
                                stop=(d == 8 and p == 3),
                                perf_mode=DR)
                    # blend: out = ps2*(1-m)*C1 + fg*m/G, mask broadcast
                    # and fg chunk fetched per chunk
                    mbc = bl.tile([128, CS], BF, tag="mbc")
                    nc.gpsimd.partition_broadcast(
                        mbc[:], mrow_bf[:, j * CS:(j + 1) * CS])
                    fgc = bl.tile([NCH, CS], F32, tag="fgc")
                    nc.sync.dma_start(fgc[:], fg_ext[:, j * CS:(j + 1) * CS])
                    imc = bl.tile([128, CS], BF, tag="imc")
                    nc.vector.tensor_scalar(imc[:], mbc[:], -C1, C1,
                                            op0=Alu.mult, op1=Alu.add)
                    t0 = bl.tile([128, CS], BF, tag="t0")
                    nc.vector.tensor_mul(t0[:], ps2[:], imc[:])
                    mfc = bl.tile([128, CS], BF, tag="mfc")
                    nc.vector.scalar_tensor_tensor(
                        mfc[:], fgc[:], 1.0 / G, mbc[:],
                        op0=Alu.mult, op1=Alu.mult)
                    outb = bl.tile([128, CS], BF, tag="outb")
                    nc.vector.tensor_add(outb[:], t0[:], mfc[:])
                    # RS pairs for chunks (0,1),(2,3),(4,5); singles for 6,7
                    # so the final RS is small and fires early
                    if j < 6:
                        if j % 2 == 0:
                            canvas = dram.tile([NCH, 2 * CS], BF, tag="cv")
                            cv_pend[j] = canvas
                        else:
                            canvas = cv_pend.pop(j - 1)
                        nc.sync.dma_start(
                            canvas[:, (j % 2) * CS:(j % 2 + 1) * CS],
                            outb[:])
                        if j % 2 == 1:
                            rs_o = dram.tile([NCH // G, 2 * CS], BF,
                                             tag="rso")
                            nc.gpsimd.collective_compute(
                                "ReduceScatter", Alu.add,
                                replica_groups=groups,
                                ins=[canvas.opt()], outs=[rs_o.opt()])
                            nc.sync.dma_start(
                                out_ext[:, (j - 1) * CS:(j + 1) * CS],
                                rs_o[:])
                    else:
                        canvas = dram.tile([NCH, CS], BF, tag="cv1")
                        nc.sync.dma_start(canvas[:], outb[:])
                        rs_o = dram.tile([NCH // G, CS], BF, tag="rso1")
                        nc.gpsimd.collective_compute(
                            "ReduceScatter", Alu.add, replica_groups=groups,
                            ins=[canvas.opt()], outs=[rs_o.opt()])
                        nc.sync.dma_start(
                            out_ext[:, j * CS:(j + 1) * CS], rs_o[:])

                hist = {}
                for k in range(NCHUNK):
                    ets = emit_g1(k)
                    if k < 3:          # kern_lc8 taps 3k..3k+2
                        for d in range(3 * k, 3 * k + 3):
                            emit_transp(d)
                    hist[k] = (ets, emit_z_ag(k, ets))
                    if k >= DC:
                        e, a = hist.pop(k - DC)
                        emit_combine(k - DC, e, a)
                    if k >= DC + 1:
                        emit_g2_blend(k - DC - 1)
                # drain
                for j in range(NCHUNK - DC, NCHUNK):
                    e, a = hist.pop(j)
                    emit_combine(j, e, a)
                    emit_g2_blend(j - 1)
                emit_g2_blend(NCHUNK - 1)

    nc.compile()
    return nc


def _shard_inputs(fg, mk):
    """fg [2,128,64,64] f32, mk [2,1,64,64] f32 -> per-core input maps."""
    in_maps = []
    for core in range(NCORES):
        b, r = core // G, core % G
        y0 = r * (W // G)
        feat = np.ascontiguousarray(fg[b].reshape(NCH, S), np.float32)
        mask = np.ascontiguousarray(mk[b].reshape(1, S), np.float32)
        band = np.zeros((NCH, 18, H), np.float32)
        mband = np.zeros((1, 18, H), np.float32)
        lo = y0 - 1
        src_lo = max(0, lo)
        src_hi = min(W, y0 + 17)
        band[:, src_lo - lo:src_hi - lo] = fg[b][:, src_lo:src_hi]
        mband[:, src_lo - lo:src_hi - lo] = mk[b][:, src_lo:src_hi]
        in_maps.append({
            "fg": feat,
            "fgband": np.ascontiguousarray(band.reshape(NCH, 18 * H)),
            "mask": mask,
            "maskband": np.ascontiguousarray(mband.reshape(1, 18 * H)),
        })
    return in_maps


def kernel(foreground, masks):
    global LAST_EXEC_TIME_NS
    from concourse.bass_utils import run_bass_kernel_spmd

    fg = np.asarray(foreground, np.float32)
    mk = np.asarray(masks, np.float32)
    assert fg.shape == (B, NCH, W, H) and mk.shape == (B, 1, W, H)

    nc = _CACHE.get("nc")
    if nc is None:
        nc = _build()
        _CACHE["nc"] = nc

    in_maps = _shard_inputs(fg, mk)
    trace = bool(os.environ.get("BASS_KERNEL_TRACE"))
    res = run_bass_kernel_spmd(nc, in_maps, core_ids=list(range(NCORES)),
                               trace=trace)
    LAST_EXEC_TIME_NS = res.exec_time_ns
    if res.exec_time_ns is not None:
        print(f"HW exec time: {res.exec_time_ns} ns")

    out = np.empty((B, NCH, W, H), np.float32)
    for core in range(NCORES):
        b, r = core // G, core % G
        out[b, 32 * r:32 * (r + 1)] = (
            res.results[core]["out"].astype(np.float32).reshape(32, W, H))
    return out


# revision 9
# speedup vs baseline: 1.0796x; 1.0796x over previous
"""Distributed Trainium2 Bass kernel for the contextual-attention module.

Strategy: data-parallel over batch (2 samples x 4 cores); within a sample
the L=4096 patch/kernel axis is sharded 4 ways (1024 kernels per core).

v3 restructure (from the v2 baseline, driven by the NTFF trace):
  * GEMM1 tap-pairs read boxbf8 DIRECTLY via hand-built strided APs
    (pair = [128][2 taps @ delta][8 rows @ 66][64 cols]) - the nine 512KB
    SBUF->SBUF pre-shift DMAs (54us serial stall) are gone.
  * The static softmax reference M~ rides in the 5th DoubleRow pass's
    padding slot as a rank-1 term: stationary block 9 = -8 on channel
    row 0, moving block = m~/8 (fp8, per-column).  The per-tile vector
    subtract (64 x [128,512]) and the msbc broadcast are gone; exp reads
    the score PSUM directly.
  * Big zero-fills via bitcast-u32 views (4x fewer elements) instead of
    35us of fp8 gpsimd memsets.
  * Kernel norms via box-filter-of-squares (2 colsum matmuls) instead of
    18 Square+matmul pairs; kern_lc8 built by fp8 PE transposes of the
    already-normalized kernT8 (batched 8-per-PSUM-bank, one copy per tap).
  * Blend maps im = (1-m)*C1 and mf = fg*m/G precomputed once; GEMM2
    accumulates all 9 taps in ONE PSUM bank; blend = mul (vector) +
    add (gpsimd); ReduceScatter + output store per chunk.
"""

import os
import sys
import types

for _p in ("/opt/trn_rl_repo",):
    if os.path.isdir(_p) and _p not in sys.path:
        sys.path.append(_p)


def _ensure_axon_hooks():
    try:
        import antenv.axon_hooks  # noqa: F401
        return
    except Exception:
        pass
    try:
        import antenv
        mod = types.ModuleType("antenv.axon_hooks")
        mod._hook = None

        def set_axon_ntff_profile_hook(hook):
            mod._hook = hook

        def get_axon_ntff_profile_hook():
            return mod._hook

        mod.set_axon_ntff_profile_hook = set_axon_ntff_profile_hook
        mod.get_axon_ntff_profile_hook = get_axon_ntff_profile_hook
        sys.modules["antenv.axon_hooks"] = mod
        antenv.axon_hooks = mod
    except Exception:
        pass


_ensure_axon_hooks()

import numpy as np  # noqa: E402

NCH = 128           # channels
W = H = 64          # spatial
S = W * H           # 4096 spatial positions
B = 2               # batch
G = 4               # cores per sample
NCORES = 8
LS = S // G         # kernels per core (1024)
LT = LS // 128      # l-tiles per core (8)
ROWS = 8            # patch-center rows per chunk
CS = ROWS * H       # spatial chunk (512)
NCHUNK = W // ROWS  # 8 chunks

KS = 16.0           # kernel scale folded into kernT8 (power of 2)
AS = 128.0          # attn scale folded into fac (fp8e4 max normal ~240)
BETA = 0.6          # softmax reference = BETA * Cauchy bound
QSLOP = 1.16        # fp8 quantization slop on the Cauchy bound
MS_C = 8.0          # m~ rides fp8 as m~/MS_C; stationary row carries -MS_C
C1 = 1.0 / (9.0 * KS * AS)   # blend scale for the GEMM2 PSUM

DC = 4              # combine lookahead (chunks); gemm2 lookahead = DC+1

_CACHE = {}
LAST_EXEC_TIME_NS = None


def _build():
    from concourse import bacc, tile, mybir
    from concourse.ap import AP
    from concourse.masks import make_identity

    F32 = mybir.dt.float32
    BF = mybir.dt.bfloat16
    F8 = mybir.dt.float8e4
    U32 = mybir.dt.uint32
    Alu = mybir.AluOpType
    Act = mybir.ActivationFunctionType
    AxX = mybir.AxisListType.X
    DR = mybir.MatmulPerfMode.DoubleRow

    nc = bacc.Bacc("TRN2", target_bir_lowering=False, debug=False,
                   num_devices=NCORES)

    fg_ext = nc.dram_tensor("fg", [NCH, S], F32, kind="ExternalInput")
    fgband_ext = nc.dram_tensor("fgband", [NCH, 18 * H], F32,
                                kind="ExternalInput")
    mask_ext = nc.dram_tensor("mask", [1, S], F32, kind="ExternalInput")
    mband_ext = nc.dram_tensor("maskband", [1, 18 * H], F32,
                               kind="ExternalInput")
    out_ext = nc.dram_tensor("out", [NCH // G, S], BF, kind="ExternalOutput")

    groups = [[0, 1, 2, 3], [4, 5, 6, 7]]
    # DoubleRow tap pairs (d0,d1) as (base_offset, delta) in the 66-wide
    # padded boxbf8 element space; tap d=(dy,dx) sits at offset dy*66+dx.
    PAIRS = [(0, 1), (2, 64), (67, 1), (132, 1)]   # taps (0,1),(2,3),(4,5),(6,7)
    MSF = KS * BETA * QSLOP

    with tile.TileContext(nc) as tc:
        with tc.tile_pool(name="const", bufs=1) as cpool, \
             tc.tile_pool(name="pers", bufs=1) as pers, \
             tc.tile_pool(name="ets", bufs=DC + 1) as etp, \
             tc.tile_pool(name="st", bufs=2) as st, \
             tc.tile_pool(name="bl", bufs=2) as bl, \
             tc.tile_pool(name="dram", bufs=3, space="DRAM") as dram, \
             tc.tile_pool(name="dramP", bufs=1, space="DRAM") as dramP:

            ident8 = cpool.tile([128, 128], F8, tag="id8")
            make_identity(nc, ident8[:])
            ones_cb = cpool.tile([128, 1], BF, tag="ones")
            nc.gpsimd.memset(ones_cb[:], 1.0)

            # ---------------- persistent tensors ----------------
            boxbf8 = pers.tile([NCH, 66, 66], F8, tag="boxbf8")
            a_flat = pers.tile([128, LT * 66 * 66], F8, tag="afull")
            af4 = a_flat[:].rearrange("c (t y x) -> c t y x", t=LT, y=66)
            kt_flat = pers.tile([NCH, 10 * LS], F8, tag="kernT8")
            kv = kt_flat[:].rearrange("c (d l) -> c d l", d=10)
            klc_flat = pers.tile([128, 9 * LT * 128], F8, tag="kernlc8")
            klcv = klc_flat[:].rearrange("l (d t c) -> l d t c", d=9, t=LT)
            mp_flat = pers.tile([128, 2 * 64 * 64], F8, tag="mpair")
            mpv = mp_flat[:].rearrange("c (b y x) -> c b y x", b=2, y=64)
            mrow_bf = pers.tile([1, S], BF, tag="mrowbf")

            bar_in = dramP.tile([16], F32, tag="bari")
            bar_out = dramP.tile([16 * NCORES], F32, tag="baro")
            bar2_in = dramP.tile([16], F32, tag="bari2")
            bar2_out = dramP.tile([16 * G], F32, tag="baro2")
            ssq_d = dramP.tile([66 * 66], F32, tag="ssqd")

            # big zero fills, issued first (no input deps; overlap DMAs):
            # a_full is fully zeroed once - combine() writes every interior
            # cell over the 8 chunks, the 1-px frame must stay 0 for GEMM2.
            # Split across three engines via u32 views.
            au = a_flat[:].bitcast(U32)
            third = 8712 // 3
            nc.vector.memset(au[:, 0:third], 0)
            nc.scalar.memzero(au[:, third:2 * third])
            nc.gpsimd.memset(au[:, 2 * third:8712], 0)
            # m~ moving block: channels 1..127 must be zero (channel 0 gets
            # the m~ row via DMA afterwards)
            nc.scalar.memzero(mp_flat[:, 64 * 64:2 * 64 * 64])
            # stationary block 9: zero except channel row 0 = -MS_C
            nc.scalar.memzero(kt_flat[:, 9 * LS:10 * LS])
            nc.gpsimd.memset(kv[0:1, 9, :], -MS_C)

            # ------------ prep: IO pool spans both stages ---------------
            pio_cm = tc.tile_pool(name="prepio", bufs=1)
            pio = pio_cm.__enter__()
            fgtmp = pio.tile([NCH, W, H], F32, tag="fgtmp")
            m2d = pio.tile([64, 64], F32, tag="m2d")
            m2d8 = pio.tile([64, 64], BF, tag="m2d8")
            mband_row = pio.tile([1, 18 * H], F32, tag="mbandrow")
            fgband_sb = pio.tile([NCH, 18, H], F32, tag="fgband")
            nc.sync.dma_start(mband_row[:], mband_ext[:])
            # warm-up barrier: triggered by the very first tiny load so
            # the rendezvous starts immediately
            nc.sync.dma_start(bar_in[:], mband_row[0:1, 0:16])
            nc.gpsimd.collective_compute(
                "AllGather", Alu.bypass,
                replica_groups=[list(range(NCORES))],
                ins=[bar_in.opt()], outs=[bar_out.opt()])
            nc.sync.dma_start(bar2_in[:], mband_row[0:1, 16:32])
            nc.gpsimd.collective_compute(
                "AllGather", Alu.bypass, replica_groups=groups,
                ins=[bar2_in.opt()], outs=[bar2_out.opt()])
            nc.sync.dma_start(
                fgband_sb[:],
                fgband_ext[:].rearrange("c (r x) -> c r x", r=18))
            fg3 = fg_ext[:].rearrange("c (y x) -> c y x", y=W)
            nc.scalar.dma_start(fgtmp[:, 0:32, :], fg3[:, 0:32, :])
            nc.gpsimd.dma_start(fgtmp[:, 32:64, :], fg3[:, 32:64, :])
            nc.sync.dma_start(m2d[:],
                              mask_ext[:].rearrange("o (p x) -> (o p) x",
                                                    p=64))
            nc.vector.tensor_copy(m2d8[:], m2d[:])

            # ------------ prep stage 1: kernels + norms (band path) -----
            with tc.tile_pool(name="prep1", bufs=1) as prep, \
                 tc.tile_pool(name="psP1", bufs=2, space="PSUM") as psP:
                # masked band -> bgbandp (zero-padded cols), bf16
                mband_bc = prep.tile([NCH, 18 * H], F32, tag="mbandbc")
                nc.gpsimd.partition_broadcast(mband_bc[:], mband_row[:])
                bgbandp = prep.tile([NCH, 18, 66], BF, tag="bgbandp")
                nc.scalar.memzero(bgbandp[:])
                nc.vector.tensor_mul(
                    bgbandp[:, :, 1:65], fgband_sb[:],
                    mband_bc[:].rearrange("c (r x) -> c r x", r=18))

                # kernel norms via box-filter of squares
                bgsq = prep.tile([NCH, 18, 66], BF, tag="bgsq")
                nc.scalar.activation(
                    bgsq[:].rearrange("c r x -> c (r x)"),
                    bgbandp[:].rearrange("c r x -> c (r x)"), Act.Square)
                hsum = prep.tile([NCH, 18, 64], BF, tag="hsum")
                nc.vector.tensor_add(hsum[:], bgsq[:, :, 0:64],
                                     bgsq[:, :, 1:65])
                nc.vector.tensor_add(hsum[:], hsum[:], bgsq[:, :, 2:66])
                nrm2 = prep.tile([NCH, 16, 64], BF, tag="mbandbc")
                nc.vector.tensor_add(nrm2[:], hsum[:, 0:16], hsum[:, 1:17])
                nc.vector.tensor_add(nrm2[:], nrm2[:], hsum[:, 2:18])
                # reciprocal on a [1, N] single-partition row is slow
                # (~6.4 ns/elem); split halves across vector and gpsimd
                norm_row = prep.tile([1, LS], F32, tag="normrow")
                for h in range(2):
                    ps_n = psP.tile([1, 512], F32, tag="psn")
                    nc.tensor.matmul(ps_n[:], ones_cb[:],
                                     nrm2[:, 8 * h:8 * h + 8, :],
                                     start=True, stop=True)
                    nc.scalar.activation(norm_row[:, 512 * h:512 * (h + 1)],
                                         ps_n[:], Act.Sqrt,
                                         scale=1.0 / (KS * KS))
                rnorm_row = prep.tile([1, LS], F32, tag="rnormrow")
                nc.vector.reciprocal(rnorm_row[:], norm_row[:])
                rnorm_bf = prep.tile([1, LS], BF, tag="rnormbf")
                nc.scalar.activation(rnorm_bf[:], rnorm_row[:], Act.Identity)
                rnorm_bc = prep.tile([NCH, LS], BF, tag="rnormbc")
                nc.gpsimd.partition_broadcast(rnorm_bc[:], rnorm_bf[:])
                rnorm3 = rnorm_bc[:].rearrange("c (r x) -> c r x", r=16)

                # normalized fp8 kernels, one strided mul per tap
                for d in range(9):
                    dy, dx = d // 3, d % 3
                    eng = nc.vector if d < 5 else nc.gpsimd
                    eng.tensor_mul(
                        kv[:, d, :].rearrange("c (r x) -> c r x", r=16),
                        bgbandp[:, dy:dy + 16, dx:dx + 64], rnorm3)

            # ------------ prep stage 2: boxfeat + M~ --------------------
            with tc.tile_pool(name="prep2", bufs=1) as prep, \
                 tc.tile_pool(name="psP2", bufs=2, space="PSUM") as psP:
                nc.sync.dma_start(mrow_bf[:], m2d8[:])

                # box-filtered feature map (3x3 sums, 66x66 padded grid)
                boxbf = prep.tile([NCH, 66, 66], BF, tag="boxbf")
                hp = prep.tile([NCH, W, 63], BF, tag="hvp")
                nc.vector.tensor_add(hp[:, 0:18, :], fgtmp[:, 0:18, 0:63],
                                     fgtmp[:, 0:18, 1:64])
                tmpH = prep.tile([NCH, W, 66], BF, tag="tmpH")
                nc.vector.tensor_add(tmpH[:, 0:18, 2:64], hp[:, 0:18, 0:62],
                                     fgtmp[:, 0:18, 2:64])
                nc.vector.tensor_add(hp[:, 18:64, :], fgtmp[:, 18:64, 0:63],
                                     fgtmp[:, 18:64, 1:64])
                nc.vector.tensor_add(tmpH[:, 18:64, 2:64], hp[:, 18:64, 0:62],
                                     fgtmp[:, 18:64, 2:64])
                nc.vector.tensor_copy(tmpH[:, :, 0:1], fgtmp[:, :, 0:1])
                nc.vector.tensor_copy(tmpH[:, :, 1:2], hp[:, :, 0:1])
                nc.vector.tensor_copy(tmpH[:, :, 64:65], hp[:, :, 62:63])
                nc.vector.tensor_copy(tmpH[:, :, 65:66], fgtmp[:, :, 63:64])
                vp = prep.tile([NCH, 63, 66], BF, tag="hvp")
                nc.vector.tensor_add(vp[:, 0:17, :], tmpH[:, 0:17, :],
                                     tmpH[:, 1:18, :])
                nc.vector.tensor_add(boxbf[:, 2:18, :], vp[:, 0:16, :],
                                     tmpH[:, 2:18, :])
                nc.vector.tensor_copy(boxbf[:, 0:1, :], tmpH[:, 0:1, :])
                nc.vector.tensor_copy(boxbf[:, 1:2, :], vp[:, 0:1, :])
                nc.vector.tensor_add(vp[:, 17:63, :], tmpH[:, 17:63, :],
                                     tmpH[:, 18:64, :])
                nc.vector.tensor_add(boxbf[:, 18:64, :], vp[:, 16:62, :],
                                     tmpH[:, 18:64, :])
                nc.vector.tensor_copy(boxbf[:, 64:65, :], vp[:, 62:63, :])
                nc.vector.tensor_copy(boxbf[:, 65:66, :], tmpH[:, 63:64, :])
                nc.scalar.activation(
                    boxbf8[:].rearrange("c y x -> c (y x)"),
                    boxbf[:].rearrange("c y x -> c (y x)"), Act.Identity)

                # m~ moving block 0 = tap-8 window of boxbf8
                nc.vector.tensor_copy(mpv[:, 0, :, :], boxbf8[:, 2:66, 2:66])

                # ---- static softmax reference M~ (Cauchy bound) ----
                boxsq = prep.tile([NCH, 66, 66], BF, tag="boxsq")
                nc.vector.tensor_mul(
                    boxsq[:].rearrange("c y x -> c (y x)"),
                    boxbf[:].rearrange("c y x -> c (y x)"),
                    boxbf[:].rearrange("c y x -> c (y x)"))
                r0s = list(range(0, 63, 7)) + [63]
                for r0 in r0s:
                    n = min(7, 66 - r0)
                    ps_q = psP.tile([1, 512], F32, tag="psq")
                    nc.tensor.matmul(
                        ps_q[:, 0:n * 66], ones_cb[:],
                        boxsq[:, r0:r0 + n, :],
                        start=True, stop=True)
                    sq_sb = prep.tile([1, 512], F32, tag="sqsb")
                    nc.scalar.activation(sq_sb[:, 0:n * 66],
                                         ps_q[:, 0:n * 66], Act.Identity)
                    nc.sync.dma_start(ssq_d[r0 * 66:(r0 + n) * 66],
                                      sq_sb[:, 0:n * 66])
                sshift = []
                for i in range(3):
                    s_i = prep.tile([64, 66], F32, tag=f"sshift{i}")
                    nc.sync.dma_start(
                        s_i[:], ssq_d[i * 66:(i + 64) * 66]
                        .rearrange("(p x) -> p x", p=64))
                    sshift.append(s_i)
                vq = prep.tile([64, 66], F32, tag="vq")
                nc.vector.tensor_add(vq[:], sshift[0][:], sshift[1][:])
                nc.vector.tensor_add(vq[:], vq[:], sshift[2][:])
                hq = prep.tile([64, 64], F32, tag="hq")
                nc.vector.tensor_add(hq[:], vq[:, 0:64], vq[:, 1:65])
                nc.vector.tensor_add(hq[:], hq[:], vq[:, 2:66])
                ms8 = prep.tile([64, 64], F8, tag="ms8")
                msf8 = MSF / MS_C
                nc.scalar.activation(ms8[:], hq[:], Act.Sqrt,
                                     scale=msf8 * msf8)
                nc.sync.dma_start(mpv[0:1, 1, :, :], ms8[:])

            pio_cm.__exit__(None, None, None)

            # ---------------- pipelined chunk loop ----------------
            bb = boxbf8[:]
            bb_part = list(bb.ap[0])     # [partition_stride, 128]

            def g1_moving(k, p):
                base, delta = PAIRS[p]
                return AP(bb.tensor, bb.offset + base + k * ROWS * 66,
                          [bb_part, [delta, 2], [66, ROWS], [1, 64]])

            with tc.tile_pool(name="psA", bufs=4, space="PSUM") as psA, \
                 tc.tile_pool(name="psG2", bufs=1, space="PSUM") as psG2, \
                 tc.tile_pool(name="psT", bufs=2, space="PSUM") as psT:

                def emit_g1(k):
                    """fp8 DR score GEMM (incl. rank-1 m~ tap) + exp."""
                    ets = []
                    for t in range(LT):
                        ps = psA.tile([128, CS], F32, tag="psA")
                        for p in range(4):
                            nc.tensor.matmul(
                                ps[:],
                                kv[:, 2 * p:2 * p + 2,
                                   t * 128:(t + 1) * 128],
                                g1_moving(k, p),
                                start=(p == 0), stop=False, perf_mode=DR)
                        nc.tensor.matmul(
                            ps[:],
                            kv[:, 8:10, t * 128:(t + 1) * 128],
                            mpv[:, :, k * ROWS:(k + 1) * ROWS, :],
                            start=False, stop=True, perf_mode=DR)
                        et = etp.tile([128, CS], BF, tag=f"et{t}")
                        nc.scalar.activation(et[:], ps[:], Act.Exp,
                                             scale=1.0 / KS)
                        ets.append(et)
                    return ets

                def emit_transp(d):
                    """kern_lc8 tap d: 8 fp8 PE transposes + one copy.
                    fp8 transpose output must land at element step 2."""
                    pt = psT.tile([128, LT, 128, 2], F8, tag="psT")
                    for t in range(LT):
                        nc.tensor.transpose(
                            pt[:, t, :, 0], kv[:, d, t * 128:(t + 1) * 128],
                            ident8[:])
                    nc.vector.tensor_copy(klcv[:, d, :, :], pt[:, :, :, 0])

                def emit_z_ag(k, ets):
                    """Z partial sums off the PE: pairwise bf16 tree over
                    the 8 et tiles on vector, then a cross-partition
                    all-reduce (f32 internally) on gpsimd."""
                    za = st.tile([128, CS], BF, tag="za")
                    zb = st.tile([128, CS], BF, tag="zb")
                    nc.vector.tensor_add(za[:], ets[0][:], ets[1][:])
                    for i in range(1, 4):
                        nc.vector.tensor_add(zb[:], ets[2 * i][:],
                                             ets[2 * i + 1][:])
                        nc.vector.tensor_add(za[:], za[:], zb[:])
                    zred = st.tile([128, CS], F32, tag="zred")
                    from concourse import bass_isa
                    nc.gpsimd.partition_all_reduce(
                        zred[:], za[:], 128, bass_isa.ReduceOp.add)
                    ag_in = dram.tile([CS], F32, tag="agi")
                    nc.sync.dma_start(ag_in[:], zred[0:1, :])
                    ag_out = dram.tile([CS * G], F32, tag="ago")
                    nc.gpsimd.collective_compute(
                        "AllGather", Alu.bypass, replica_groups=groups,
                        ins=[ag_in.opt()], outs=[ag_out.opt()])
                    return ag_out

                def emit_combine(k, ets, ag_out):
                    """global Z -> fac = AS/Z broadcast; a8 = et * fac."""
                    zz = st.tile([32, G, CS // 32], F32, tag="zz")
                    nc.sync.dma_start(
                        zz[:], ag_out[:].rearrange("(r p i) -> p r i",
                                                   r=G, p=32))
                    gs = st.tile([32, CS // 32], F32, tag="gs")
                    nc.vector.tensor_reduce(
                        gs[:], zz[:].rearrange("p r i -> p i r"), AxX,
                        Alu.add)
                    rg = st.tile([32, CS // 32], F32, tag="rg")
                    nc.vector.reciprocal(rg[:], gs[:])
                    fac32 = st.tile([32, CS // 32], BF, tag="fac32")
                    nc.vector.tensor_scalar_mul(fac32[:], rg[:], AS)
                    fac_row = st.tile([1, CS], BF, tag="facrow")
                    nc.sync.dma_start(fac_row[:], fac32[:])
                    fac_bc = st.tile([128, CS], BF, tag="facbc")
                    nc.gpsimd.partition_broadcast(fac_bc[:], fac_row[:])
                    fb3 = fac_bc[:].rearrange("p (y x) -> p y x", y=ROWS)
                    for t in range(LT):
                        nc.vector.tensor_mul(
                            af4[:, t, 1 + k * ROWS:1 + (k + 1) * ROWS, 1:65],
                            ets[t][:].rearrange("p (y x) -> p y x", y=ROWS),
                            fb3)

                cv_pend = {}

                def emit_g2_blend(j):
                    """fp8 transpose-conv, all 9 taps in one PSUM bank,
                    then blend; ReduceScatter every two chunks."""
                    ps2 = psG2.tile([128, CS], F32, tag="psG2")
                    for d in range(9):
                        dy, dx = d // 3, d % 3
                        sy0 = j * ROWS + 2 - dy
                        sx0 = 2 - dx
                        for p in range(4):
                            nc.tensor.matmul(
                                ps2[:],
                                klcv[:, d, 2 * p:2 * p + 2, :],
                                af4[:, 2 * p:2 * p + 2,
                                    sy0:sy0 + ROWS, sx0:sx0 + 64],
                                start=(d == 0 and p == 0),
                                stop=(d == 8 and p == 3),
                                perf_mode=DR)
                    # blend: out = ps2*(1-m)*C1 + fg*m/G, mask broadcast
                    # and fg chunk fetched per chunk
                    mbc = bl.tile([128, CS], BF, tag="mbc")
                    nc.gpsimd.partition_broadcast(
                        mbc[:], mrow_bf[:, j * CS:(j + 1) * CS])
                    fgc = bl.tile([NCH, CS], F32, tag="fgc")
                    nc.sync.dma_start(fgc[:], fg_ext[:, j * CS:(j + 1) * CS])
                    imc = bl.tile([128, CS], BF, tag="imc")
                    nc.vector.tensor_scalar(imc[:], mbc[:], -C1, C1,
                                            op0=Alu.mult, op1=Alu.add)
                    t0 = bl.tile([128, CS], BF, tag="t0")
                    nc.vector.tensor_mul(t0[:], ps2[:], imc[:])
                    mfc = bl.tile([128, CS], BF, tag="mfc")
                    nc.vector.scalar_tensor_tensor(
                        mfc[:], fgc[:], 1.0 / G, mbc[:],
                        op0=Alu.mult, op1=Alu.mult)
                    outb = bl.tile([128, CS], BF, tag="outb")
                    nc.vector.tensor_add(outb[:], t0[:], mfc[:])
                    # RS pairs for chunks (0,1),(2,3),(4,5); singles for 6,7
                    # so the final RS is small and fires early
                    if j < 6:
                        if j % 2 == 0:
                            canvas = dram.tile([NCH, 2 * CS], BF, tag="cv")
                            cv_pend[j] = canvas
                        else:
                            canvas = cv_pend.pop(j - 1)
                        nc.sync.dma_start(
                            canvas[:, (j % 2) * CS:(j % 2 + 1) * CS],
                            outb[:])
                        if j % 2 == 1:
                            rs_o = dram.tile([NCH // G, 2 * CS], BF,
                                             tag="rso")
                            nc.gpsimd.collective_compute(
                                "ReduceScatter", Alu.add,
                                replica_groups=groups,
                                ins=[canvas.opt()], outs=[rs_o.opt()])
                            nc.sync.dma_start(
                                out_ext[:, (j - 1) * CS:(j + 1) * CS],
                                rs_o[:])
                    else:
                        canvas = dram.tile([NCH, CS], BF, tag="cv1")
                        nc.sync.dma_start(canvas[:], outb[:])
                        rs_o = dram.tile([NCH // G, CS], BF, tag="rso1")
                        nc.gpsimd.collective_compute(
                            "ReduceScatter", Alu.add, replica_groups=groups,
                            ins=[canvas.opt()], outs=[rs_o.opt()])
                        nc.sync.dma_start(
                            out_ext[:, j * CS:(j + 1) * CS], rs_o[:])

                hist = {}
                for k in range(NCHUNK):
                    ets = emit_g1(k)
                    if k < 3:          # kern_lc8 taps 3k..3k+2
                        for d in range(3 * k, 3 * k + 3):
                            emit_transp(d)
                    hist[k] = (ets, emit_z_ag(k, ets))
                    if k >= DC:
                        e, a = hist.pop(k - DC)
                        emit_combine(k - DC, e, a)
                    if k >= DC + 1:
                        emit_g2_blend(k - DC - 1)
                # drain
                for j in range(NCHUNK - DC, NCHUNK):
                    e, a = hist.pop(j)
                    emit_combine(j, e, a)
                    emit_g2_blend(j - 1)
                emit_g2_blend(NCHUNK - 1)

    nc.compile()
    return nc


def _shard_inputs(fg, mk):
    """fg [2,128,64,64] f32, mk [2,1,64,64] f32 -> per-core input maps."""
    in_maps = []
    for core in range(NCORES):
        b, r = core // G, core % G
        y0 = r * (W // G)
        feat = np.ascontiguousarray(fg[b].reshape(NCH, S), np.float32)
        mask = np.ascontiguousarray(mk[b].reshape(1, S), np.float32)
        band = np.zeros((NCH, 18, H), np.float32)
        mband = np.zeros((1, 18, H), np.float32)
        lo = y0 - 1
        src_lo = max(0, lo)
        src_hi = min(W, y0 + 17)
        band[:, src_lo - lo:src_hi - lo] = fg[b][:, src_lo:src_hi]
        mband[:, src_lo - lo:src_hi - lo] = mk[b][:, src_lo:src_hi]
        in_maps.append({
            "fg": feat,
            "fgband": np.ascontiguousarray(band.reshape(NCH, 18 * H)),
            "mask": mask,
            "maskband": np.ascontiguousarray(mband.reshape(1, 18 * H)),
        })
    return in_maps


def kernel(foreground, masks):
    global LAST_EXEC_TIME_NS
    from concourse.bass_utils import run_bass_kernel_spmd

    fg = np.asarray(foreground, np.float32)
    mk = np.asarray(masks, np.float32)
    assert fg.shape == (B, NCH, W, H) and mk.shape == (B, 1, W, H)

    nc = _CACHE.get("nc")
    if nc is None:
        nc = _build()
        _CACHE["nc"] = nc

    in_maps = _shard_inputs(fg, mk)
    trace = bool(os.environ.get("BASS_KERNEL_TRACE"))
    res = run_bass_kernel_spmd(nc, in_maps, core_ids=list(range(NCORES)),
                               trace=trace)
    LAST_EXEC_TIME_NS = res.exec_time_ns
    if res.exec_time_ns is not None:
        print(f"HW exec time: {res.exec_time_ns} ns")

    out = np.empty((B, NCH, W, H), np.float32)
    for core in range(NCORES):
        b, r = core // G, core % G
        out[b, 32 * r:32 * (r + 1)] = (
            res.results[core]["out"].astype(np.float32).reshape(32, W, H))
    return out
